# revision 1
# baseline (speedup 1.0000x reference)
"""GCN encoder (2x GCNConv + BatchNorm + ReLU) on 8 Trainium2 NeuronCores.

Strategy (graph/data parallel, per sharding hint):
- Nodes are permuted (degree-sorted, round-robin dealt) and sharded across the
  8 cores; each core owns 49 "windows" of 128 destination nodes.
- norm factorizes: norm(s,d) = dis[s]*dis[d].  Source scaling dis[s] is folded
  into the feature tables (h~ = dis * h); destination scaling dis[d] is applied
  on PSUM eviction.  Messages then aggregate with a *constant identity* matmul:
  for each window, gathered source rows land in "slots" (slot = local dst id),
  and chunk matmuls with a preloaded identity accumulate them in PSUM.
- Gathers use the int16 dma_gather embedding path.  int16 limits addressing to
  32768 rows, so the feature table is split in two halves (cores 0-3 / 4-7) and
  each window runs one gather per half; pad slots point at an all-zero row.
  First/last windows split their gathers into pieces to shorten pipeline
  fill/drain; conv2 uses a second, slimmer index stream without self-loops
  (the self-loop message is the core's own h2stage row, added as one extra
  identity matmul per window).
- h1 = x @ W1 is computed redundantly on every core (full table in local HBM),
  staged and written per 7-tile group.  h2's input depends on BN1 (global
  stats); each core computes its shard of h2 = relu(bn(conv1)) @ W2 (o1 chunks
  transposed early, during conv1) and an AllGather replicates the table.
- BatchNorm uses E[x^2]-mean^2 with sums computed by ones-vector matmuls
  (partition reduction) accumulated in PSUM across windows; cross-core stats
  reduce via a small AllGather + on-chip sum (cheaper than AllReduce, which
  the cost model prices at 1.875x).  BN1 factor math runs feature-on-partition
  (transposed); BN2 apply runs as batched bf16 DVE ops (2x mode) with an
  activation-engine f32 convert streamed out per group.
- b1/b2 are ignored: a per-feature constant added before BatchNorm cancels
  exactly in (x - mean).
"""

import sys

sys.path.insert(0, "/opt/trn_rl_repo")

import numpy as np

N_CORES = 8
P = 128
EPS = 1e-5

_FULL_CFG = dict(N=50000, IN=512, D1=256, D2=128)


# ---------------------------------------------------------------- host preprocessing

def _preprocess(edge_index, N):
    """Graph preprocessing: node permutation, slot/chunk assignment, gather
    indices.  Pure integer work on the host."""
    src = np.asarray(edge_index[0], dtype=np.int64)
    dst = np.asarray(edge_index[1], dtype=np.int64)
    # append self loops
    loop = np.arange(N, dtype=np.int64)
    S = np.concatenate([src, loop])
    D = np.concatenate([dst, loop])

    deg = np.bincount(D, minlength=N)  # >= 1 (self loop)
    dis = (1.0 / np.sqrt(deg.astype(np.float64))).astype(np.float32)

    real_pc = N // N_CORES
    WPC = (real_pc + P - 1) // P          # windows per core
    SLOTS = WPC * P                        # slot positions per core
    BLK = SLOTS + 1                        # +1 trailing zero row per core block

    # deal nodes to cores round-robin in degree-desc order
    order = np.argsort(-deg, kind="stable")
    core_of = np.empty(N, dtype=np.int64)
    core_of[order] = np.arange(N) % N_CORES

    half_node = core_of >= (N_CORES // 2)  # False = lo table half
    halfE = half_node[S]

    deg_lo = np.bincount(D[~halfE], minlength=N)
    deg_hi = deg - deg_lo

    # position of each node within its core: sort by (deg_lo desc, deg_hi desc),
    # then re-sort blocks of 768 by deg_hi — keeps per-window max(deg_lo) and
    # max(deg_hi) both tight (gather padding ~18% instead of ~31%)
    pos = np.empty(N, dtype=np.int64)
    node_by_cp = np.full((N_CORES, SLOTS), -1, dtype=np.int64)
    RESORT_BLK = 896
    for c in range(N_CORES):
        nodes_c = np.flatnonzero(core_of == c)
        o = np.lexsort((-deg_hi[nodes_c], -deg_lo[nodes_c]))
        for s in range(0, len(o), RESORT_BLK):
            blk = o[s : s + RESORT_BLK]
            o[s : s + RESORT_BLK] = blk[np.argsort(-deg_hi[nodes_c][blk], kind="stable")]
        snodes = nodes_c[o]
        pos[snodes] = np.arange(len(snodes))
        node_by_cp[c, : len(snodes)] = snodes

    # per-core per-window chunk counts -> global max (SPMD static shapes)
    dlo_cp = np.zeros((N_CORES, SLOTS), dtype=np.int64)
    dhi_cp = np.zeros((N_CORES, SLOTS), dtype=np.int64)
    m = node_by_cp >= 0
    dlo_cp[m] = deg_lo[node_by_cp[m]]
    dhi_cp[m] = deg_hi[node_by_cp[m]]
    NLc = dlo_cp.reshape(N_CORES, WPC, P).max(axis=2)
    NHc = dhi_cp.reshape(N_CORES, WPC, P).max(axis=2)
    NL = NLc.max(axis=0)
    NH = NHc.max(axis=0)

    def build_idx(S_, D_, halfE_, NL_, NH_):
        """Gather index stream: per window [lo seg][hi seg], chunk-major."""
        seg_ = (NL_ + NH_) * P
        base_ = np.concatenate([[0], np.cumsum(seg_)])
        offL_ = base_[:-1]
        offH_ = base_[:-1] + NL_ * P
        TOT_ = int(base_[-1])

        # chunk index of each edge among its (dst, half) group
        key = D_ * 2 + halfE_
        ksort = np.argsort(key, kind="stable")
        skey = key[ksort]
        starts = np.concatenate([[0], np.flatnonzero(np.diff(skey)) + 1])
        group_len = np.diff(np.concatenate([starts, [len(skey)]]))
        chunk_sorted = np.arange(len(skey)) - np.repeat(starts, group_len)
        chunk = np.empty(len(S_), dtype=np.int64)
        chunk[ksort] = chunk_sorted

        cD = core_of[D_]
        wD = pos[D_] // P
        slotD = pos[D_] % P
        absrow = core_of[S_] * BLK + pos[S_]
        rel = np.where(~halfE_, absrow, absrow - (N_CORES // 2) * BLK)
        assert rel.max() < 32768
        epos = np.where(~halfE_, offL_[wD], offH_[wD]) + chunk * P + slotD

        PADIDX = SLOTS  # each block's trailing zero row (rel within half view)
        flat = np.full(N_CORES * TOT_, PADIDX, dtype=np.int16)
        flat[cD * TOT_ + epos] = rel.astype(np.int16)
        flat = flat.reshape(N_CORES, TOT_)
        # wrap: idx i -> [i%16, i//16], replicated across the 8 groups of 16
        wrapped16 = flat.reshape(N_CORES, TOT_ // 16, 16).transpose(0, 2, 1)
        return np.tile(wrapped16, (1, P // 16, 1)), offL_, offH_, TOT_

    idx_wrapped, offL, offH, TOT = build_idx(S, D, halfE, NL, NH)

    # conv2 stream: real edges only (self-loop handled as a direct h2stage
    # matmul); same windows, own chunk maxima
    realE = len(src)
    S2, D2_, halfE2 = S[:realE], D[:realE], halfE[:realE]
    dlo2 = np.bincount(D2_[~halfE2], minlength=N)
    dhi2 = np.bincount(D2_[halfE2], minlength=N)
    d2lo_cp = np.zeros((N_CORES, SLOTS), dtype=np.int64)
    d2hi_cp = np.zeros((N_CORES, SLOTS), dtype=np.int64)
    d2lo_cp[m] = dlo2[node_by_cp[m]]
    d2hi_cp[m] = dhi2[node_by_cp[m]]
    NL2 = d2lo_cp.reshape(N_CORES, WPC, P).max(axis=2).max(axis=0)
    NH2 = d2hi_cp.reshape(N_CORES, WPC, P).max(axis=2).max(axis=0)
    idx2_wrapped, offL2, offH2, TOT2 = build_idx(S2, D2_, halfE2, NL2, NH2)

    # per-core dis (by slot), 1.0 for dummies
    dis_cp = np.ones((N_CORES, SLOTS), dtype=np.float32)
    dis_cp[m] = dis[node_by_cp[m]]
    dismy = dis_cp.reshape(N_CORES, WPC, P).transpose(0, 2, 1)  # [c, 128, WPC]

    # global dis in table-data-row order (for phase 1), [128, NTILES]
    NTILES = N_CORES * WPC
    disall = dis_cp.reshape(NTILES, P).T.copy()  # [128, NTILES]

    # stats mask: last window has (SLOTS - real_pc) dummy rows at the end
    n_dummy = SLOTS - real_pc
    statmask = np.ones((P, 2), dtype=np.float32)
    if n_dummy:
        statmask[P - n_dummy :, 1] = 0.0

    waste = float(TOT) / max(1, len(S) / N_CORES) - 1.0
    return dict(
        WPC=WPC, SLOTS=SLOTS, BLK=BLK, NTILES=NTILES,
        NL=NL.astype(int), NH=NH.astype(int), TOT=TOT,
        offL=offL, offH=offH,
        NL2=NL2.astype(int), NH2=NH2.astype(int), TOT2=TOT2,
        offL2=offL2, offH2=offH2, idx2_wrapped=idx2_wrapped,
        idx_wrapped=idx_wrapped, dismy=dismy, disall=disall,
        statmask=statmask, node_by_cp=node_by_cp, pos=pos, core_of=core_of,
        dis=dis, real_pc=real_pc, waste=waste,
    )


def _pack_inputs(x, W1, W2, pp, cfg):
    """Build the per-core / shared device input arrays."""
    import ml_dtypes

    bf16 = ml_dtypes.bfloat16
    N, IN, D1, D2 = cfg["N"], cfg["IN"], cfg["D1"], cfg["D2"]
    WPC, SLOTS, NTILES = pp["WPC"], pp["SLOTS"], pp["NTILES"]
    KC = IN // P

    # x permuted to table order (pre-scaled by dis so h~ = (dis*x) @ W1),
    # zeros for dummies -> [NTILES, 128(p=k), KC, 128(j=row)]
    xperm = np.zeros((N_CORES * SLOTS, IN), dtype=np.float32)
    m = pp["node_by_cp"] >= 0
    xperm[m.reshape(-1)] = x[pp["node_by_cp"][m]] * pp["dis"][pp["node_by_cp"][m]][:, None]
    xb = (
        xperm.reshape(NTILES, P, KC, P)   # [b, j, kc, p]
        .transpose(0, 3, 2, 1)            # [b, p, kc, j]
        .astype(bf16)
    )
    w1b = W1.reshape(KC, P, D1).transpose(1, 0, 2).astype(bf16)   # [p, kc, D1]
    w2b = W2.reshape(D1 // P, P, D2).transpose(1, 0, 2).astype(bf16)  # [p, kc, D2]
    return xb, w1b, w2b


# ---------------------------------------------------------------- device kernel

def _build_kernel(cfg, pp, phases=5):
    import concourse.bacc as bacc
    import concourse.mybir as mybir
    import concourse.tile as tile
    from concourse.masks import make_identity
    from contextlib import ExitStack

    N, IN, D1, D2 = cfg["N"], cfg["IN"], cfg["D1"], cfg["D2"]
    WPC, SLOTS, BLK, NTILES = pp["WPC"], pp["SLOTS"], pp["BLK"], pp["NTILES"]
    NL, NH, TOT = pp["NL"], pp["NH"], pp["TOT"]
    offL, offH = pp["offL"], pp["offH"]
    NL2, NH2, TOT2 = pp["NL2"], pp["NH2"], pp["TOT2"]
    offL2, offH2 = pp["offL2"], pp["offH2"]
    KC = IN // P
    KC2 = D1 // P
    HB = (N_CORES // 2) * BLK        # hi half base row
    NROWS = N_CORES * BLK            # table rows
    NTmax = int((NL + NH).max())
    NT2max = int((NL2 + NH2).max())
    RG = [list(range(N_CORES))]
    f32, bf16, i16 = mybir.dt.float32, mybir.dt.bfloat16, mybir.dt.int16
    f32r = mybir.dt.float32r
    AF = mybir.ActivationFunctionType

    nc = bacc.Bacc(num_devices=N_CORES)

    # ---- I/O
    xb_d = nc.dram_tensor("xb", [NTILES, P, KC, P], bf16, kind="ExternalInput")
    w1_d = nc.dram_tensor("w1b", [P, KC, D1], bf16, kind="ExternalInput")
    w2_d = nc.dram_tensor("w2b", [P, KC2, D2], bf16, kind="ExternalInput")
    idx_d = nc.dram_tensor("idx", [P, TOT // 16], i16, kind="ExternalInput")
    idx2_d = nc.dram_tensor("idx2", [P, TOT2 // 16], i16, kind="ExternalInput")
    dismy_d = nc.dram_tensor("dismy", [P, WPC], f32, kind="ExternalInput")
    mask_d = nc.dram_tensor("statmask", [P, 2], f32, kind="ExternalInput")
    g1_d = nc.dram_tensor("gamma1", [1, D1], f32, kind="ExternalInput")
    b1_d = nc.dram_tensor("beta1", [1, D1], f32, kind="ExternalInput")
    g2_d = nc.dram_tensor("gamma2", [1, D2], f32, kind="ExternalInput")
    b2_d = nc.dram_tensor("beta2", [1, D2], f32, kind="ExternalInput")
    out_d = nc.dram_tensor("out", [SLOTS, D2], f32, kind="ExternalOutput")

    # ---- internal DRAM
    h1tab = nc.dram_tensor("h1tab", [NROWS, D1], bf16, kind="Internal")
    h2shard = nc.dram_tensor("h2shard", [BLK, D2], bf16, kind="Internal")
    h2tab = nc.dram_tensor("h2tab", [NROWS, D2], bf16, kind="Internal", addr_space="Shared")
    ar1_in = nc.dram_tensor("ar1_in", [1, 2 * D1], f32, kind="Internal")
    ar1_out = nc.dram_tensor("ar1_out", [N_CORES, 2 * D1], f32, kind="Internal", addr_space="Shared")
    ar2_in = nc.dram_tensor("ar2_in", [1, 2 * D2], f32, kind="Internal")
    ar2_out = nc.dram_tensor("ar2_out", [N_CORES, 2 * D2], f32, kind="Internal", addr_space="Shared")

    import concourse.bass as bass

    def pad_rows_ap(tensor, D):
        # rows {c*BLK + SLOTS : c in 0..7} of a [NROWS, D] table
        return bass.AP(tensor, SLOTS * D, [[BLK * D, N_CORES], [1, D]])

    with tile.TileContext(nc) as tc:
        es = ExitStack()
        with es:
            cpool = es.enter_context(tc.tile_pool(name="const", bufs=1))
            ident_b = cpool.tile([P, P], bf16)
            make_identity(nc, ident_b[:])
            ident_f = cpool.tile([P, P], f32)
            make_identity(nc, ident_f[:])
            w1_s = cpool.tile([P, KC, D1], bf16)
            nc.sync.dma_start(out=w1_s[:], in_=w1_d[:, :, :])
            w2_s = cpool.tile([P, KC2, D2], bf16)
            nc.sync.dma_start(out=w2_s[:], in_=w2_d[:, :, :])
            dismy_s = cpool.tile([P, WPC], f32)
            nc.scalar.dma_start(out=dismy_s[:], in_=dismy_d[:, :])
            mask_s = cpool.tile([P, 2], f32)
            nc.scalar.dma_start(out=mask_s[:], in_=mask_d[:, :])
            mask_b = cpool.tile([P, 2], bf16)
            nc.vector.tensor_copy(out=mask_b[:], in_=mask_s[:])
            gb_s = cpool.tile([1, 2 * D1 + 2 * D2], f32)  # gamma1|beta1|gamma2|beta2
            nc.scalar.dma_start(out=gb_s[:, 0:D1], in_=g1_d[:, :])
            nc.scalar.dma_start(out=gb_s[:, D1 : 2 * D1], in_=b1_d[:, :])
            nc.scalar.dma_start(out=gb_s[:, 2 * D1 : 2 * D1 + D2], in_=g2_d[:, :])
            nc.scalar.dma_start(out=gb_s[:, 2 * D1 + D2 :], in_=b2_d[:, :])
            # gamma1/beta1 transposed ([P, KC2] feature-on-partition), done
            # here while PE is idle; consumed by the BN1 factor block
            gbT = cpool.tile([P, 2, KC2], f32)
            with tc.tile_pool(name="gbtp", bufs=4, space="PSUM") as gbtp:
                for i in range(2):
                    for c in range(KC2):
                        tp = gbtp.tile([P, 1], f32, tag="gbt")
                        nc.tensor.transpose(
                            out=tp[:], in_=gb_s[:, (i * KC2 + c) * P : (i * KC2 + c + 1) * P],
                            identity=ident_f[0:1, 0:1],
                        )
                        nc.vector.tensor_copy(out=gbT[:, i, c : c + 1], in_=tp[:])

            # zero pad rows of h1tab (one strided DMA)
            zrow = cpool.tile([N_CORES, D1], bf16)
            nc.vector.memset(zrow[:], 0)
            nc.gpsimd.dma_start(out=pad_rows_ap(h1tab, D1), in_=zrow[:])

            # conv1 long-lived tiles: opened before phase 1 so the idx load
            # DMA issues at the head of the stream
            es1 = ExitStack()  # conv1/BN1/h2-phase pools, closed before conv2
            o1_pool = es1.enter_context(tc.tile_pool(name="o1", bufs=1))
            o1_all = o1_pool.tile([P, WPC, D1], bf16)
            idx_s = o1_pool.tile([P, TOT // 16], i16)
            # o1T allocated last: the conv2 pool then reuses the o1_all/idx1
            # region (free at conv1-end), so the idx2 load isn't anti-dep
            # gated on o1T (which lives until the last bnr op)
            o1T = o1_pool.tile([P, WPC, KC2, P], bf16)  # transposed o1 chunks
            nc.scalar.dma_start(out=idx_s[:], in_=idx_d[:, :])

            # ---------------- phase 1: h1tab = dis * (x @ W1), all rows ----------------
            with (
                tc.tile_pool(name="p1x", bufs=6) as xpool,
                tc.tile_pool(name="p1s", bufs=8) as spool,
                tc.tile_pool(name="p1p", bufs=6, space="PSUM") as ppool1,
            ):
                XB = 7 if WPC % 7 == 0 else 1   # x tiles per DMA + stage write
                units = []
                for blk in range(N_CORES):
                    for tb in range(WPC // XB):
                        units.append((blk, tb * XB, XB))
                # split the final unit so the trailing PE/write chain is short
                blk_l, t0_l, n_l = units.pop()
                units.extend([(blk_l, t0_l, XB - 1), (blk_l, t0_l + XB - 1, 1)])
                for blk, t0, nx in units:
                    b0 = blk * WPC + t0
                    xt = xpool.tile([P, XB, KC, P], bf16, tag="xt")
                    nc.sync.dma_start(
                        out=xt[:, 0:nx], in_=xb_d[b0 : b0 + nx].rearrange("b p k j -> p b k j")
                    )
                    stage = spool.tile([P, XB, D1], bf16, tag="stage")
                    for t2 in range(nx):
                        ps = ppool1.tile([P, D1], f32, tag="ps1")
                        for kc in range(KC):
                            nc.tensor.matmul(
                                out=ps[:], lhsT=xt[:, t2, kc, :], rhs=w1_s[:, kc, :],
                                start=(kc == 0), stop=(kc == KC - 1),
                            )
                        if t2 % 2 == 0:
                            nc.scalar.activation(
                                stage[:, t2, :], ps[:], AF.Copy
                            )
                        else:
                            nc.vector.tensor_copy(out=stage[:, t2, :], in_=ps[:])
                    r0 = blk * BLK + t0 * P
                    nc.gpsimd.dma_start(
                        out=h1tab[r0 : r0 + nx * P, :].rearrange(
                            "(t p) d -> p t d", p=P
                        ),
                        in_=stage[:, 0:nx],
                    )

            # ---------------- conv1: window aggregation ----------------
            if phases < 2:
                outst = cpool.tile([P, WPC, D2], f32)
                nc.vector.memset(outst[:], 0)
                nc.gpsimd.dma_start(
                    out=out_d[0:SLOTS, :].rearrange("(t p) d -> p t d", p=P),
                    in_=outst[:],
                )
                return nc
            lo_view = h1tab[0:HB, :]
            hi_view = h1tab[HB:NROWS, :]
            with (
                tc.tile_pool(name="g1", bufs=3) as gpool,
                tc.tile_pool(name="sq1", bufs=4) as sqpool,
                tc.tile_pool(name="c1p", bufs=4, space="PSUM") as wpool,
                tc.tile_pool(name="st1p", bufs=1, space="PSUM") as stpool,
                tc.tile_pool(name="trc", bufs=2, space="PSUM") as trcpool,
            ):
                st_s = stpool.tile([1, D1], f32, tag="st_s")
                st_q = stpool.tile([1, D1], f32, tag="st_q")
                for w in range(WPC):
                    nl, nh = int(NL[w]), int(NH[w])
                    nt = nl + nh
                    gb = gpool.tile([P, NTmax, D1], bf16, tag="g1")
                    # split the last windows' gathers so the PE tail after the
                    # final piece is short
                    pieces = 3 if w == WPC - 1 else 2 if (w == WPC - 2 or w == 0) else 1
                    for n0, base, view, off in ((nl, 0, lo_view, offL[w]),
                                                (nh, nl, hi_view, offH[w])):
                        if not n0:
                            continue
                        cuts = [n0 * i // pieces for i in range(pieces + 1)]
                        for a, b in zip(cuts[:-1], cuts[1:]):
                            if b == a:
                                continue
                            nc.gpsimd.dma_gather(
                                gb[:, base + a : base + b, :], view,
                                idx_s[:, off // 16 + a * 8 : off // 16 + b * 8],
                                (b - a) * P, (b - a) * P, D1,
                                elem_step=D1, single_packet=False,
                            )
                    ps = wpool.tile([P, D1], f32, tag="win1")
                    for j in range(nt):
                        nc.tensor.matmul(
                            out=ps[:], lhsT=ident_b[:], rhs=gb[:, j, :],
                            start=(j == 0), stop=(j == nt - 1),
                        )
                    nc.vector.tensor_scalar_mul(
                        o1_all[:, w, :], ps[:], dismy_s[:, w : w + 1]
                    )
                    sq = sqpool.tile([P, D1], bf16, tag="sq")
                    nc.vector.tensor_mul(sq[:], o1_all[:, w, :], o1_all[:, w, :])
                    mcol = mask_b[:, 1:2] if w == WPC - 1 else mask_b[:, 0:1]
                    nc.tensor.matmul(
                        out=st_s[:], lhsT=mcol, rhs=o1_all[:, w, :],
                        start=(w == 0), stop=(w == WPC - 1), skip_group_check=True,
                    )
                    nc.tensor.matmul(
                        out=st_q[:], lhsT=mcol, rhs=sq[:],
                        start=(w == 0), stop=(w == WPC - 1), skip_group_check=True,
                    )
                    # transpose o1 chunks now (PE+ACT are underused here) so
                    # the post-AllReduce h2 phase is activation+matmul only
                    for c in range(KC2):
                        tpc = trcpool.tile([P, P], bf16, tag="tpc")
                        nc.tensor.transpose(
                            out=tpc[:], in_=o1_all[:, w, c * P : (c + 1) * P],
                            identity=ident_b[:],
                        )
                        nc.scalar.activation(o1T[:, w, c, :], tpc[:], AF.Copy)
                # stats -> DRAM -> AllReduce
                stats1 = o1_pool.tile([1, 2 * D1], f32)
                nc.vector.tensor_copy(out=stats1[:, 0:D1], in_=st_s[:])
                nc.vector.tensor_copy(out=stats1[:, D1:], in_=st_q[:])
            if phases < 3:
                outst = cpool.tile([P, WPC, D2], f32)
                nc.vector.tensor_copy(out=outst[:], in_=o1_all[:, :, 0:D2])
                nc.gpsimd.dma_start(
                    out=out_d[0:SLOTS, :].rearrange("(t p) d -> p t d", p=P),
                    in_=outst[:],
                )
                return nc
            nc.gpsimd.dma_start(out=ar1_in[:, :], in_=stats1[:])
            nc.gpsimd.collective_compute(
                "AllGather", mybir.AluOpType.bypass,
                ins=[ar1_in[:, :]], outs=[ar1_out[:, :]], replica_groups=RG,
            )

            # ---------------- BN1 factors + h2 shard ----------------
            bnp = es1.enter_context(tc.tile_pool(name="bn1", bufs=1))
            sg8 = bnp.tile([N_CORES, 2 * D1], f32)
            nc.sync.dma_start(out=sg8[:], in_=ar1_out[:, :])

            ones8 = bnp.tile([N_CORES, 1], f32)
            nc.vector.memset(ones8[:], 1.0)
            sg = bnp.tile([1, 2 * D1], f32)
            with tc.tile_pool(name="sgp", bufs=1, space="PSUM") as sgpool:
                sgps = sgpool.tile([1, 2 * D1], f32, tag="sgps")
                nc.tensor.matmul(out=sgps[:], lhsT=ones8[:], rhs=sg8[:], start=True, stop=True)
                nc.vector.tensor_copy(out=sg[:], in_=sgps[:])
            # transpose the sums first, then do all factor math feature-on-
            # partition at [P, KC2] — single-partition [1, D1] ops are ~2x
            # slower and would need post-hoc transposes anyway
            sgT = bnp.tile([P, 2, KC2], f32)  # [:,0,:] sums, [:,1,:] sumsq
            with tc.tile_pool(name="trp", bufs=4, space="PSUM") as trpool:
                for i in range(2):
                    for c in range(KC2):
                        tp = trpool.tile([P, 1], f32, tag="tr")
                        nc.tensor.transpose(
                            out=tp[:], in_=sg[:, (i * KC2 + c) * P : (i * KC2 + c + 1) * P],
                            identity=ident_f[0:1, 0:1],
                        )
                        nc.vector.tensor_copy(out=sgT[:, i, c : c + 1], in_=tp[:])
            mqT = bnp.tile([P, 2, KC2], f32)
            nc.vector.tensor_scalar_mul(mqT[:], sgT[:], 1.0 / N)
            meanT = mqT[:, 0, :]
            varT = bnp.tile([P, KC2], f32)
            nc.vector.tensor_mul(varT[:], meanT, meanT)
            nc.vector.tensor_sub(varT[:], mqT[:, 1, :], varT[:])
            epstP = bnp.tile([P, 1], f32)
            nc.vector.memset(epstP[:], EPS)
            sdT = bnp.tile([P, KC2], f32)
            nc.scalar.activation(sdT[:], varT[:], AF.Sqrt, bias=epstP[:])
            rstdT = bnp.tile([P, KC2], f32)
            nc.vector.reciprocal(rstdT[:], sdT[:])
            acT = bnp.tile([P, KC2, 2], f32)
            nc.vector.tensor_mul(acT[:, :, 0], rstdT[:], gbT[:, 0, :])
            tmpT = bnp.tile([P, KC2], f32)
            nc.vector.tensor_mul(tmpT[:], meanT, acT[:, :, 0])
            nc.vector.tensor_sub(acT[:, :, 1], gbT[:, 1, :], tmpT[:])

            # per 7-window group: transpose o1 chunks, one batched BN+ReLU
            # activation per (group, chunk), then per-window W2 matmuls
            GH = 7
            assert WPC % GH == 0
            # h2stage doubles as the conv2 self-loop rhs; keep it in the
            # long-lived const pool so it survives es1.close()
            h2stage = cpool.tile([P, WPC, D2], bf16)
            with (
                tc.tile_pool(name="bnr", bufs=10) as bpool,
                tc.tile_pool(name="h2p", bufs=8, space="PSUM") as h2pool,
            ):
                h2ap = h2shard[0:SLOTS, :].rearrange("(t p) d -> p t d", p=P)
                for g in range(WPC // GH):
                    ws = g * GH
                    bnrs = []
                    for c in range(KC2):
                        # bnr = relu(a1*o1T + c1) as two DVE bf16 ops (scalar
                        # operands are per-partition f32, exempt from 2x mode)
                        bnr = bpool.tile([P, GH, P], bf16, tag=f"bnr{c}")
                        nc.vector.tensor_scalar(
                            bnr[:], o1T[:, ws : ws + GH, c, :],
                            acT[:, c, 0:1], acT[:, c, 1:2],
                            mybir.AluOpType.mult, mybir.AluOpType.add,
                        )
                        nc.vector.tensor_scalar_max(bnr[:], bnr[:], 0.0)
                        bnrs.append(bnr)
                    for t in range(GH):
                        h2ps = h2pool.tile([P, D2], f32, tag="h2ps")
                        for c in range(KC2):
                            nc.tensor.matmul(
                                out=h2ps[:], lhsT=bnrs[c][:, t, :], rhs=w2_s[:, c, :],
                                start=(c == 0), stop=(c == KC2 - 1),
                            )
                        if (ws + t) % 4 == 3:
                            nc.vector.tensor_scalar_mul(
                                h2stage[:, ws + t, :], h2ps[:],
                                dismy_s[:, ws + t : ws + t + 1],
                            )
                        else:
                            nc.scalar.activation(
                                h2stage[:, ws + t, :], h2ps[:], AF.Copy,
                                scale=dismy_s[:, ws + t : ws + t + 1],
                            )
                    # stream this group's h2shard rows out now so the
                    # AllGather isn't gated on one big trailing write
                    nc.sync.dma_start(
                        out=h2ap[:, ws : ws + GH, :],
                        in_=h2stage[:, ws : ws + GH, :],
                    )
            zrow2 = bnp.tile([1, D2], bf16)
            nc.vector.memset(zrow2[:], 0)
            nc.sync.dma_start(out=h2shard[SLOTS:BLK, :], in_=zrow2[:])
            nc.gpsimd.collective_compute(
                "AllGather", mybir.AluOpType.bypass,
                ins=[h2shard[:, :]], outs=[h2tab[:, :]], replica_groups=RG,
            )
            if phases < 4:
                outst = cpool.tile([P, WPC, D2], f32)
                nc.vector.tensor_copy(out=outst[:], in_=h2stage[:])
                nc.gpsimd.dma_start(
                    out=out_d[0:SLOTS, :].rearrange("(t p) d -> p t d", p=P),
                    in_=outst[:],
                )
                return nc

            # ---------------- conv2 ----------------
            es1.close()  # free o1_all / bn1-phase SBUF before conv2
            o2_pool = es.enter_context(tc.tile_pool(name="o2", bufs=1))
            o2_all = o2_pool.tile([P, WPC, D2], bf16)
            idx2_s = o2_pool.tile([P, TOT2 // 16], i16)
            nc.sync.dma_start(out=idx2_s[:], in_=idx2_d[:, :])
            lo2 = h2tab[0:HB, :]
            hi2 = h2tab[HB:NROWS, :]
            with (
                tc.tile_pool(name="g2", bufs=5) as gpool2,
                tc.tile_pool(name="sq2", bufs=6) as sqpool2,
                tc.tile_pool(name="c2p", bufs=6, space="PSUM") as wpool2,
                tc.tile_pool(name="st2p", bufs=1, space="PSUM") as stpool2,
            ):
                st2_s = stpool2.tile([1, D2], f32, tag="st2_s")
                st2_q = stpool2.tile([1, D2], f32, tag="st2_q")
                for w in range(WPC):
                    nl, nh = int(NL2[w]), int(NH2[w])
                    nt = nl + nh
                    gb = gpool2.tile([P, NT2max, D2], bf16, tag="g2")
                    pieces = 3 if w == WPC - 1 else 2 if (w == WPC - 2 or w == 0) else 1
                    for n0, base, view, off in ((nl, 0, lo2, offL2[w]),
                                                (nh, nl, hi2, offH2[w])):
                        if not n0:
                            continue
                        cuts = [n0 * i // pieces for i in range(pieces + 1)]
                        for a, b in zip(cuts[:-1], cuts[1:]):
                            if b == a:
                                continue
                            nc.gpsimd.dma_gather(
                                gb[:, base + a : base + b, :], view,
                                idx2_s[:, off // 16 + a * 8 : off // 16 + b * 8],
                                (b - a) * P, (b - a) * P, D2,
                                elem_step=D2, single_packet=False,
                            )
                    ps = wpool2.tile([P, D2], f32, tag="win2")
                    # self-loop: h2stage row w IS dis*h2 for my slots; issue
                    # first so PE starts before the gather lands
                    nc.tensor.matmul(
                        out=ps[:], lhsT=ident_b[:], rhs=h2stage[:, w, :],
                        start=True, stop=(nt == 0),
                    )
                    for j in range(nt):
                        nc.tensor.matmul(
                            out=ps[:], lhsT=ident_b[:], rhs=gb[:, j, :],
                            start=False, stop=(j == nt - 1),
                        )
                    nc.vector.tensor_scalar_mul(
                        o2_all[:, w, :], ps[:], dismy_s[:, w : w + 1]
                    )
                    sq = sqpool2.tile([P, D2], bf16, tag="sq2")
                    nc.vector.tensor_mul(sq[:], o2_all[:, w, :], o2_all[:, w, :])
                    mcol = mask_b[:, 1:2] if w == WPC - 1 else mask_b[:, 0:1]
                    nc.tensor.matmul(
                        out=st2_s[:], lhsT=mcol, rhs=o2_all[:, w, :],
                        start=(w == 0), stop=(w == WPC - 1), skip_group_check=True,
                    )
                    nc.tensor.matmul(
                        out=st2_q[:], lhsT=mcol, rhs=sq[:],
                        start=(w == 0), stop=(w == WPC - 1), skip_group_check=True,
                    )
                stats2 = o2_pool.tile([1, 2 * D2], f32)
                nc.vector.tensor_copy(out=stats2[:, 0:D2], in_=st2_s[:])
                nc.vector.tensor_copy(out=stats2[:, D2:], in_=st2_q[:])
            if phases < 5:
                outst = cpool.tile([P, WPC, D2], f32)
                nc.vector.tensor_copy(out=outst[:], in_=o2_all[:, :, :])
                nc.gpsimd.dma_start(
                    out=out_d[0:SLOTS, :].rearrange("(t p) d -> p t d", p=P),
                    in_=outst[:],
                )
                return nc
            nc.gpsimd.dma_start(out=ar2_in[:, :], in_=stats2[:])
            nc.gpsimd.collective_compute(
                "AllGather", mybir.AluOpType.bypass,
                ins=[ar2_in[:, :]], outs=[ar2_out[:, :]], replica_groups=RG,
            )

            # ---------------- BN2 + output ----------------
            sg28 = o2_pool.tile([N_CORES, 2 * D2], f32)
            nc.sync.dma_start(out=sg28[:], in_=ar2_out[:, :])
            ones82 = o2_pool.tile([N_CORES, 1], f32)
            nc.vector.memset(ones82[:], 1.0)
            mq2 = o2_pool.tile([1, 2 * D2], f32)
            with tc.tile_pool(name="sg2p", bufs=1, space="PSUM") as sg2pool:
                sg2ps = sg2pool.tile([1, 2 * D2], f32, tag="sg2ps")
                nc.tensor.matmul(out=sg2ps[:], lhsT=ones82[:], rhs=sg28[:], start=True, stop=True)
                # one fused scale straight from PSUM: [mean2 | ex22]
                nc.vector.tensor_scalar_mul(mq2[:], sg2ps[:], 1.0 / N)
            mean2 = mq2[:, 0:D2]
            ex22 = mq2[:, D2:]
            var2 = o2_pool.tile([1, D2], f32)
            nc.vector.tensor_mul(var2[:], mean2, mean2)
            nc.vector.tensor_sub(var2[:], ex22, var2[:])
            epst2 = o2_pool.tile([1, 1], f32)
            nc.vector.memset(epst2[:], EPS)
            sd2 = o2_pool.tile([1, D2], f32)
            nc.scalar.activation(sd2[:], var2[:], AF.Sqrt, bias=epst2[:])
            rstd2 = o2_pool.tile([1, D2], f32)
            nc.vector.reciprocal(rstd2[:], sd2[:])
            a2 = o2_pool.tile([1, D2], f32)
            nc.vector.tensor_mul(a2[:], rstd2[:], gb_s[:, 2 * D1 : 2 * D1 + D2])
            c2 = o2_pool.tile([1, D2], f32)
            nc.vector.tensor_mul(c2[:], mean2, a2[:])
            nc.vector.tensor_sub(c2[:], gb_s[:, 2 * D1 + D2 :], c2[:])

            # broadcast a2/c2 across partitions, tiled GW-wide so the BN2
            # apply runs on [P, GW*D2] batches
            GW = 7
            assert WPC % GW == 0
            onesrow = o2_pool.tile([1, P], bf16)
            nc.vector.memset(onesrow[:], 1.0)
            ac2row = o2_pool.tile([1, 2, GW, D2], bf16)
            for b, srct in ((0, a2), (1, c2)):
                nc.vector.tensor_copy(out=ac2row[:, b, 0, :], in_=srct[:])
                done = 1
                while done < GW:
                    n = min(done, GW - done)
                    nc.vector.tensor_copy(
                        out=ac2row[:, b, done : done + n, :],
                        in_=ac2row[:, b, 0:n, :],
                    )
                    done += n
            acb = o2_pool.tile([P, 2, GW, D2], bf16)
            acb_flat = acb[:].rearrange("p b c d -> p (b c d)")
            ac2_flat = ac2row[:].rearrange("a b c d -> a (b c d)")
            CHK = 448  # <= 512 f32 per PSUM bank
            with tc.tile_pool(name="bn2p", bufs=4, space="PSUM") as bn2p:
                for i in range(0, 2 * GW * D2, CHK):
                    bps = bn2p.tile([P, CHK], f32, tag="b2a")
                    nc.tensor.matmul(out=bps[:], lhsT=onesrow[:],
                                     rhs=ac2_flat[:, i : i + CHK],
                                     start=True, stop=True)
                    nc.scalar.activation(acb_flat[:, i : i + CHK], bps[:], AF.Copy)

            outst = o2_pool.tile([P, WPC, D2], bf16)
            outap = out_d[0:SLOTS, :].rearrange("(t p) d -> p t d", p=P)
            with tc.tile_pool(name="of32", bufs=8) as ofpool:
                ngroups = WPC // GW
                spans = [(g * GW, GW) for g in range(ngroups - 1)]
                spans += [((ngroups - 1) * GW + t, 1) for t in range(GW)]
                for s0, n in spans:
                    sl = slice(s0, s0 + n)
                    nc.vector.tensor_mul(outst[:, sl, :], o2_all[:, sl, :], acb[:, 0, 0:n])
                    nc.vector.tensor_add(outst[:, sl, :], outst[:, sl, :], acb[:, 1, 0:n])
                    of32 = ofpool.tile([P, GW, D2], f32, tag="of32")
                    nc.scalar.activation(of32[:, 0:n], outst[:, sl, :], AF.Copy)
                    nc.sync.dma_start(out=outap[:, sl, :], in_=of32[:, 0:n])

    return nc


# ---------------------------------------------------------------- entry point

def _run(x, edge_index, W1, gamma1, beta1, W2, gamma2, beta2, cfg, trace=False):
    from concourse.bass_utils import run_bass_kernel_spmd

    N = cfg["N"]
    pp = _preprocess(edge_index, N)
    xb, w1b, w2b = _pack_inputs(np.asarray(x, np.float32), np.asarray(W1, np.float32),
                                np.asarray(W2, np.float32), pp, cfg)
    nc = _build_kernel(cfg, pp, phases=int(__import__("os").environ.get("K_PHASES", "5")))
    nc.compile()

    shared = {
        "xb": np.ascontiguousarray(xb),
        "w1b": np.ascontiguousarray(w1b),
        "w2b": np.ascontiguousarray(w2b),
        "statmask": np.ascontiguousarray(pp["statmask"]),
        "gamma1": np.asarray(gamma1, np.float32).reshape(1, -1),
        "beta1": np.asarray(beta1, np.float32).reshape(1, -1),
        "gamma2": np.asarray(gamma2, np.float32).reshape(1, -1),
        "beta2": np.asarray(beta2, np.float32).reshape(1, -1),
    }
    in_maps = []
    for c in range(N_CORES):
        m = dict(shared)
        m["idx"] = np.ascontiguousarray(pp["idx_wrapped"][c])
        m["idx2"] = np.ascontiguousarray(pp["idx2_wrapped"][c])
        m["dismy"] = np.ascontiguousarray(pp["dismy"][c])
        in_maps.append(m)

    res = run_bass_kernel_spmd(nc, in_maps, core_ids=list(range(N_CORES)), trace=trace)
    _run.last_nc = nc

    D2 = cfg["D2"]
    out = np.empty((N, D2), np.float32)
    pos, core_of = pp["pos"], pp["core_of"]
    for c in range(N_CORES):
        nodes = np.flatnonzero(core_of == c)
        out[nodes] = res.results[c]["out"][pos[nodes]]
    _run.last_result = res
    return out


def kernel(x, edge_index, W1, b1, gamma1, beta1, W2, b2, gamma2, beta2):
    # b1/b2 cancel exactly through BatchNorm's mean subtraction; unused.
    return _run(x, edge_index, W1, gamma1, beta1, W2, gamma2, beta2, _FULL_CFG)



# revision 3
# speedup vs baseline: 1.0425x; 1.0425x over previous
"""GCN encoder (2x GCNConv + BatchNorm + ReLU) on 8 Trainium2 NeuronCores.

Strategy (graph/data parallel, per sharding hint):
- Nodes are permuted (degree-sorted, round-robin dealt) and sharded across the
  8 cores; each core owns 49 "windows" of 128 destination nodes.
- conv1 aggregates FIRST, transforms SECOND: out1 = ((D^-.5 A D^-.5) x) @ W1.
  The gathered table is dis*x quantized to fp8e4m3 (512 B rows -> the DMA
  cost model's <512B 2x latency multiplier doesn't apply), pre-scaled by a
  power-of-two s for fp8 range; s is divided back out (folded into the dis[d]
  eviction scale).  There is no per-core x@W1 phase at all - the host ships
  the fp8 table directly.
- Messages aggregate with constant identity matmuls: gathered source rows
  land in "slots" (slot = local dst id), and chunk matmuls accumulate them in
  PSUM.  fp8 chunks use MatmulPerfMode.DoubleRow with a stacked [I;I] weight
  to sum TWO chunks per instruction at 0.5 cycles/row.
- After aggregation each window is dis[d]-scaled, transposed (4 PE
  transposes), and multiplied by W1 halves - producing o1T (features on
  partitions) directly, which is exactly the layout BatchNorm factors and the
  h2 = relu(bn(o1)) @ W2 stage consume.  BN1 statistics are free-dim DVE
  reductions of o1T accumulated in SBUF; cross-core reduction is a tiny
  [128,4] AllGather + on-chip sum.
- Self loops never enter the gather stream: the self message in x-space is
  the core's own table row, kept in SBUF (xown) and added as one extra
  matmul per window.  conv2's self loop is the core's own h2stage row
  (as before).
- conv1 and conv2 share ONE index stream: both tables use the same
  [8 x (SLOTS+1)] row layout (trailing zero row per core block), and both
  convs aggregate over the same real-edge set, so the int16 gather indices
  (split in lo/hi table halves for int16 range) are identical.
- h2 = relu(bn1(o1)) @ W2 is computed per-shard and replicated via AllGather
  (bf16; fp8 doesn't help conv2's gather due to the 256B elem-size floor).
- b1/b2 are ignored: a per-feature constant added before BatchNorm cancels
  exactly in (x - mean).
"""

import sys

sys.path.insert(0, "/opt/trn_rl_repo")

import numpy as np

N_CORES = 8
P = 128
EPS = 1e-5

_FULL_CFG = dict(N=50000, IN=512, D1=256, D2=128)


# ---------------------------------------------------------------- host preprocessing

def _preprocess(edge_index, N):
    """Graph preprocessing: node permutation, slot/chunk assignment, gather
    indices.  Pure integer work on the host."""
    src = np.asarray(edge_index[0], dtype=np.int64)
    dst = np.asarray(edge_index[1], dtype=np.int64)
    loop = np.arange(N, dtype=np.int64)
    D_all = np.concatenate([dst, loop])

    deg = np.bincount(D_all, minlength=N)  # >= 1 (self loop)
    dis = (1.0 / np.sqrt(deg.astype(np.float64))).astype(np.float32)

    real_pc = N // N_CORES
    WPC = (real_pc + P - 1) // P          # windows per core
    SLOTS = WPC * P                        # slot positions per core
    BLK = SLOTS + 1                        # +1 trailing zero row per core block

    # deal nodes to cores round-robin in degree-desc order
    order = np.argsort(-deg, kind="stable")
    core_of = np.empty(N, dtype=np.int64)
    core_of[order] = np.arange(N) % N_CORES

    half_node = core_of >= (N_CORES // 2)  # False = lo table half
    halfE = half_node[src]                 # real edges only

    deg_lo = np.bincount(dst[~halfE], minlength=N)
    deg_hi = np.bincount(dst[halfE], minlength=N)

    # position of each node within its core: sort by (deg_lo desc, deg_hi
    # desc), then re-sort blocks by deg_hi - keeps per-window max(deg_lo) and
    # max(deg_hi) both tight
    pos = np.empty(N, dtype=np.int64)
    node_by_cp = np.full((N_CORES, SLOTS), -1, dtype=np.int64)
    RESORT_BLK = 896
    for c in range(N_CORES):
        nodes_c = np.flatnonzero(core_of == c)
        o = np.lexsort((-deg_hi[nodes_c], -deg_lo[nodes_c]))
        for s in range(0, len(o), RESORT_BLK):
            blk = o[s : s + RESORT_BLK]
            o[s : s + RESORT_BLK] = blk[np.argsort(-deg_hi[nodes_c][blk], kind="stable")]
        snodes = nodes_c[o]
        pos[snodes] = np.arange(len(snodes))
        node_by_cp[c, : len(snodes)] = snodes

    # per-core per-window chunk counts -> global max (SPMD static shapes)
    dlo_cp = np.zeros((N_CORES, SLOTS), dtype=np.int64)
    dhi_cp = np.zeros((N_CORES, SLOTS), dtype=np.int64)
    m = node_by_cp >= 0
    dlo_cp[m] = deg_lo[node_by_cp[m]]
    dhi_cp[m] = deg_hi[node_by_cp[m]]
    NL = dlo_cp.reshape(N_CORES, WPC, P).max(axis=2).max(axis=0)
    NH = dhi_cp.reshape(N_CORES, WPC, P).max(axis=2).max(axis=0)

    # gather index stream: per window [lo seg][hi seg], chunk-major
    seg = (NL + NH) * P
    base = np.concatenate([[0], np.cumsum(seg)])
    offL = base[:-1]
    offH = base[:-1] + NL * P
    TOT = int(base[-1])

    # chunk index of each edge among its (dst, half) group
    key = dst * 2 + halfE
    ksort = np.argsort(key, kind="stable")
    skey = key[ksort]
    starts = np.concatenate([[0], np.flatnonzero(np.diff(skey)) + 1])
    group_len = np.diff(np.concatenate([starts, [len(skey)]]))
    chunk_sorted = np.arange(len(skey)) - np.repeat(starts, group_len)
    chunk = np.empty(len(src), dtype=np.int64)
    chunk[ksort] = chunk_sorted

    cD = core_of[dst]
    wD = pos[dst] // P
    slotD = pos[dst] % P
    absrow = core_of[src] * BLK + pos[src]
    rel = np.where(~halfE, absrow, absrow - (N_CORES // 2) * BLK)
    assert rel.max() < 32768
    epos = np.where(~halfE, offL[wD], offH[wD]) + chunk * P + slotD

    PADIDX = SLOTS  # block 0's trailing zero row (rel within half view)
    flat = np.full(N_CORES * TOT, PADIDX, dtype=np.int16)
    flat[cD * TOT + epos] = rel.astype(np.int16)
    flat = flat.reshape(N_CORES, TOT)
    # wrap: idx i -> [i%16, i//16], replicated across the 8 groups of 16
    wrapped16 = flat.reshape(N_CORES, TOT // 16, 16).transpose(0, 2, 1)
    idx_wrapped = np.tile(wrapped16, (1, P // 16, 1))

    # per-core dis (by slot), 1.0 for dummies
    dis_cp = np.ones((N_CORES, SLOTS), dtype=np.float32)
    dis_cp[m] = dis[node_by_cp[m]]
    dismy = dis_cp.reshape(N_CORES, WPC, P).transpose(0, 2, 1)  # [c, 128, WPC]

    # stats mask for conv2 (dummy slots have nonzero h2 = relu(bn1(0)))
    n_dummy = SLOTS - real_pc
    statmask = np.ones((P, 2), dtype=np.float32)
    if n_dummy:
        statmask[P - n_dummy :, 1] = 0.0

    waste = float(TOT) / max(1, len(src) / N_CORES) - 1.0
    return dict(
        WPC=WPC, SLOTS=SLOTS, BLK=BLK,
        NL=NL.astype(int), NH=NH.astype(int), TOT=TOT,
        offL=offL, offH=offH, idx_wrapped=idx_wrapped,
        dismy=dismy, statmask=statmask,
        node_by_cp=node_by_cp, pos=pos, core_of=core_of,
        dis=dis, real_pc=real_pc, waste=waste,
    )


def _pack_inputs(x, W1, W2, gamma1, beta1, pp, cfg):
    """Build the per-core / shared device input arrays."""
    import ml_dtypes

    f16 = np.float16
    f8 = ml_dtypes.float8_e3m4
    N, IN, D1, D2 = cfg["N"], cfg["IN"], cfg["D1"], cfg["D2"]
    WPC, SLOTS, BLK = pp["WPC"], pp["SLOTS"], pp["BLK"]
    KC, KC2 = IN // P, D1 // P
    NROWS = N_CORES * BLK

    # fp8 quantization scale: power of two, headroom below e3m4 max (15.5),
    # large enough to keep the bulk of values out of the denormal zone
    v = np.asarray(x, np.float32) * pp["dis"][:, None]
    maxv = float(np.abs(v).max())
    s = float(2.0 ** np.floor(np.log2(14.0 / maxv))) if maxv > 0 else 1.0

    xtab = np.zeros((NROWS, IN), dtype=f8)
    m = pp["node_by_cp"] >= 0
    rows = (np.arange(N_CORES)[:, None] * BLK + np.arange(SLOTS)[None, :])[m]
    xtab[rows] = (v[pp["node_by_cp"][m]] * s).astype(f8)

    xown = np.empty((N_CORES, P, WPC, IN), dtype=f8)
    for c in range(N_CORES):
        xown[c] = xtab[c * BLK : c * BLK + SLOTS].reshape(WPC, P, IN).transpose(1, 0, 2)

    # dismy2[c, :, 0, :] = dis/s (conv1 eviction), [:, 1, :] = dis (h2 / conv2)
    dismy2 = np.empty((N_CORES, P, 2, WPC), dtype=np.float32)
    dismy2[:, :, 0, :] = pp["dismy"] / s
    dismy2[:, :, 1, :] = pp["dismy"]

    w1b = W1.reshape(KC, P, D1).transpose(1, 0, 2).astype(f16)        # [p, kc, D1]
    w2b = W2.reshape(KC2, P, D2).transpose(1, 0, 2).astype(f16)       # [p, kc, D2]
    g1t = np.asarray(gamma1, np.float32).reshape(KC2, P).T.copy()     # [p, kc2]
    b1t = np.asarray(beta1, np.float32).reshape(KC2, P).T.copy()

    i2 = np.zeros((P, P), dtype=f8)
    i2[np.arange(P), np.arange(P)] = 1.0
    return xtab, xown, dismy2, w1b, w2b, g1t, b1t, i2


# ---------------------------------------------------------------- device kernel

def _build_kernel(cfg, pp, phases=5):
    import concourse.bacc as bacc
    import concourse.mybir as mybir
    import concourse.tile as tile
    from concourse.masks import make_identity
    from contextlib import ExitStack

    N, IN, D1, D2 = cfg["N"], cfg["IN"], cfg["D1"], cfg["D2"]
    WPC, SLOTS, BLK = pp["WPC"], pp["SLOTS"], pp["BLK"]
    NL, NH, TOT = pp["NL"], pp["NH"], pp["TOT"]
    offL, offH = pp["offL"], pp["offH"]
    KC = IN // P
    KC2 = D1 // P
    HB = (N_CORES // 2) * BLK        # hi half base row
    NROWS = N_CORES * BLK            # table rows
    NTmax = int((NL + NH).max())
    RG = [list(range(N_CORES))]
    f32, f16, i16 = mybir.dt.float32, mybir.dt.float16, mybir.dt.int16
    f8 = mybir.dt.float8e3
    AF = mybir.ActivationFunctionType
    AX = mybir.AxisListType
    ALU = mybir.AluOpType

    nc = bacc.Bacc(num_devices=N_CORES)

    # ---- I/O
    xtab_d = nc.dram_tensor("xtab", [NROWS, IN], f8, kind="ExternalInput")
    xown_d = nc.dram_tensor("xown", [P, WPC, IN], f8, kind="ExternalInput")
    idx_d = nc.dram_tensor("idx", [P, TOT // 16], i16, kind="ExternalInput")
    dismy_d = nc.dram_tensor("dismy", [P, 2, WPC], f32, kind="ExternalInput")
    i2_d = nc.dram_tensor("i2", [P, P], f8, kind="ExternalInput")
    w1_d = nc.dram_tensor("w1b", [P, KC, D1], f16, kind="ExternalInput")
    w2_d = nc.dram_tensor("w2b", [P, KC2, D2], f16, kind="ExternalInput")
    g1t_d = nc.dram_tensor("g1t", [P, KC2], f32, kind="ExternalInput")
    b1t_d = nc.dram_tensor("b1t", [P, KC2], f32, kind="ExternalInput")
    mask_d = nc.dram_tensor("statmask", [P, 2], f32, kind="ExternalInput")
    g2_d = nc.dram_tensor("gamma2", [1, D2], f32, kind="ExternalInput")
    b2_d = nc.dram_tensor("beta2", [1, D2], f32, kind="ExternalInput")
    out_d = nc.dram_tensor("out", [SLOTS, D2], f32, kind="ExternalOutput")

    # ---- internal DRAM
    h2shard = nc.dram_tensor("h2shard", [BLK, D2], f16, kind="Internal")
    h2tab = nc.dram_tensor("h2tab", [NROWS, D2], f16, kind="Internal", addr_space="Shared")
    ar1_in = nc.dram_tensor("ar1_in", [P, 4], f32, kind="Internal")
    ar1_out = nc.dram_tensor("ar1_out", [N_CORES, P, 4], f32, kind="Internal", addr_space="Shared")
    ar2_in = nc.dram_tensor("ar2_in", [1, 2 * D2], f32, kind="Internal")
    ar2_out = nc.dram_tensor("ar2_out", [N_CORES, 2 * D2], f32, kind="Internal", addr_space="Shared")

    with tile.TileContext(nc) as tc:
        es = ExitStack()
        with es:
            cpool = es.enter_context(tc.tile_pool(name="const", bufs=1))
            # idx stream first: its load DMA heads the queue, the first
            # gather depends on it
            idx_s = cpool.tile([P, TOT // 16], i16)
            nc.sync.dma_start(out=idx_s[:], in_=idx_d[:, :])
            ident_b = cpool.tile([P, P], f16)
            make_identity(nc, ident_b[:])
            i2_s = cpool.tile([P, P], f8)
            nc.scalar.dma_start(out=i2_s[:], in_=i2_d[:, :])
            w1_s = cpool.tile([P, KC, D1], f16)
            nc.scalar.dma_start(out=w1_s[:], in_=w1_d[:, :, :])
            w2_s = cpool.tile([P, KC2, D2], f16)
            nc.scalar.dma_start(out=w2_s[:], in_=w2_d[:, :, :])
            dismy_s = cpool.tile([P, 2, WPC], f32)
            nc.scalar.dma_start(out=dismy_s[:], in_=dismy_d[:, :, :])
            g1t_s = cpool.tile([P, KC2], f32)
            nc.scalar.dma_start(out=g1t_s[:], in_=g1t_d[:, :])
            b1t_s = cpool.tile([P, KC2], f32)
            nc.scalar.dma_start(out=b1t_s[:], in_=b1t_d[:, :])
            mask_s = cpool.tile([P, 2], f32)
            nc.scalar.dma_start(out=mask_s[:], in_=mask_d[:, :])
            mask_b = cpool.tile([P, 2], f16)
            nc.vector.tensor_copy(out=mask_b[:], in_=mask_s[:])
            gb2_s = cpool.tile([1, 2 * D2], f32)  # gamma2|beta2
            nc.scalar.dma_start(out=gb2_s[:, 0:D2], in_=g2_d[:, :])
            nc.scalar.dma_start(out=gb2_s[:, D2:], in_=b2_d[:, :])
            # h2stage doubles as the conv2 self-loop rhs
            h2stage = cpool.tile([P, WPC, D2], f16)

            # zero pad row of h2shard (row SLOTS)
            zrow2 = cpool.tile([1, D2], f16)
            nc.vector.memset(zrow2[:], 0)
            nc.sync.dma_start(out=h2shard[SLOTS:BLK, :], in_=zrow2[:])

            # conv1-lifetime pools
            es1 = ExitStack()
            o1_pool = es1.enter_context(tc.tile_pool(name="o1", bufs=1))
            xown_s = o1_pool.tile([P, WPC, IN], f8)
            nc.sync.dma_start(out=xown_s[:], in_=xown_d[:, :, :])
            o1T = o1_pool.tile([P, WPC, KC2, P], f16)
            stacc = o1_pool.tile([P, WPC, 4], f32)

            # ---------------- conv1: aggregate-first windows ----------------
            lo_view = xtab_d[0:HB, :]
            hi_view = xtab_d[HB:NROWS, :]
            with (
                tc.tile_pool(name="g1", bufs=3) as gpool,
                tc.tile_pool(name="ag", bufs=4) as apool,
                tc.tile_pool(name="at", bufs=3) as atpool,
                tc.tile_pool(name="sq1", bufs=4) as sqpool,
                tc.tile_pool(name="c1p", bufs=3, space="PSUM") as wpool,
                tc.tile_pool(name="trp", bufs=2, space="PSUM") as trpool,
                tc.tile_pool(name="o1p", bufs=2, space="PSUM") as o1ppool,
            ):
                for w in range(WPC):
                    nl, nh = int(NL[w]), int(NH[w])
                    nt = nl + nh
                    gb = gpool.tile([P, NTmax, IN], f8, tag="g1")
                    pieces = 3 if w == WPC - 1 else 2 if (w == WPC - 2 or w == 0) else 1
                    for n0, bs, view, off in ((nl, 0, lo_view, offL[w]),
                                              (nh, nl, hi_view, offH[w])):
                        if not n0:
                            continue
                        cuts = [n0 * i // pieces for i in range(pieces + 1)]
                        for a, b in zip(cuts[:-1], cuts[1:]):
                            if b == a:
                                continue
                            nc.gpsimd.dma_gather(
                                gb[:, bs + a : bs + b, :], view,
                                idx_s[:, off // 16 + a * 8 : off // 16 + b * 8],
                                (b - a) * P, (b - a) * P, IN,
                                elem_step=IN, single_packet=False,
                            )
                    ps = wpool.tile([P, IN], f32, tag="win1")
                    # self loop first: starts PE before the gather lands
                    nc.tensor.matmul(
                        out=ps[:], lhsT=i2_s[:], rhs=xown_s[:, w, :],
                        start=True, stop=(nt == 0),
                    )
                    for j in range(nt):
                        nc.tensor.matmul(
                            out=ps[:], lhsT=i2_s[:], rhs=gb[:, j, :],
                            start=False, stop=(j == nt - 1),
                        )
                    # evict + dis[d]/s scale
                    agg_s = apool.tile([P, IN], f16, tag="ag")
                    nc.vector.tensor_scalar_mul(agg_s[:], ps[:], dismy_s[:, 0, w : w + 1])
                    # transpose the 4 chunks
                    aT = atpool.tile([P, KC, P], f16, tag="at")
                    for kc in range(KC):
                        tp = trpool.tile([P, P], f16, tag="tr")
                        nc.tensor.transpose(
                            out=tp[:], in_=agg_s[:, kc * P : (kc + 1) * P],
                            identity=ident_b[:],
                        )
                        nc.scalar.activation(aT[:, kc, :], tp[:], AF.Copy)
                    # o1T halves: W1^T @ agg^T
                    for h in range(KC2):
                        op = o1ppool.tile([P, P], f32, tag="o1p")
                        for kc in range(KC):
                            nc.tensor.matmul(
                                out=op[:], lhsT=w1_s[:, kc, h * P : (h + 1) * P],
                                rhs=aT[:, kc, :],
                                start=(kc == 0), stop=(kc == KC - 1),
                            )
                        nc.scalar.activation(o1T[:, w, h, :], op[:], AF.Copy)
                    # BN1 stats: free-dim reductions over slots
                    sq = sqpool.tile([P, KC2, P], f16, tag="sq")
                    nc.vector.tensor_mul(sq[:], o1T[:, w], o1T[:, w])
                    nc.vector.tensor_reduce(
                        stacc[:, w, 0:2], o1T[:, w], axis=AX.X, op=ALU.add
                    )
                    nc.vector.tensor_reduce(
                        stacc[:, w, 2:4], sq[:], axis=AX.X, op=ALU.add
                    )
                stT = o1_pool.tile([P, 4], f32)
                nc.vector.tensor_reduce(
                    stT[:], stacc[:].rearrange("p w f -> p f w"), axis=AX.X, op=ALU.add
                )
            if phases < 3:
                outst = cpool.tile([P, WPC, D2], f32)
                nc.vector.memset(outst[:], 0)
                nc.gpsimd.dma_start(
                    out=out_d[0:SLOTS, :].rearrange("(t p) d -> p t d", p=P),
                    in_=outst[:],
                )
                return nc
            nc.gpsimd.dma_start(out=ar1_in[:, :], in_=stT[:])
            nc.gpsimd.collective_compute(
                "AllGather", mybir.AluOpType.bypass,
                ins=[ar1_in[:, :]], outs=[ar1_out[:, :, :]], replica_groups=RG,
            )

            # ---------------- BN1 factors + h2 shard ----------------
            bnp = es1.enter_context(tc.tile_pool(name="bn1", bufs=1))
            sg8 = bnp.tile([P, N_CORES, 4], f32)
            nc.sync.dma_start(out=sg8[:], in_=ar1_out[:, :, :].rearrange("c p f -> p c f"))
            stT8 = bnp.tile([P, 4], f32)
            nc.vector.tensor_reduce(
                stT8[:], sg8[:].rearrange("p c f -> p f c"), axis=AX.X, op=ALU.add
            )
            mq = bnp.tile([P, 4], f32)
            nc.vector.tensor_scalar_mul(mq[:], stT8[:], 1.0 / N)
            varT = bnp.tile([P, KC2], f32)
            nc.vector.tensor_mul(varT[:], mq[:, 0:2], mq[:, 0:2])
            nc.vector.tensor_sub(varT[:], mq[:, 2:4], varT[:])
            epstP = bnp.tile([P, 1], f32)
            nc.vector.memset(epstP[:], EPS)
            sdT = bnp.tile([P, KC2], f32)
            nc.scalar.activation(sdT[:], varT[:], AF.Sqrt, bias=epstP[:])
            rstdT = bnp.tile([P, KC2], f32)
            nc.vector.reciprocal(rstdT[:], sdT[:])
            acT = bnp.tile([P, KC2, 2], f32)
            nc.vector.tensor_mul(acT[:, :, 0], rstdT[:], g1t_s[:])
            tmpT = bnp.tile([P, KC2], f32)
            nc.vector.tensor_mul(tmpT[:], mq[:, 0:2], acT[:, :, 0])
            nc.vector.tensor_sub(acT[:, :, 1], b1t_s[:], tmpT[:])

            # per 7-window group: batched BN+ReLU then per-window W2 matmuls
            GH = 7
            assert WPC % GH == 0
            with (
                tc.tile_pool(name="bnr", bufs=10) as bpool,
                tc.tile_pool(name="h2p", bufs=8, space="PSUM") as h2pool,
            ):
                h2ap = h2shard[0:SLOTS, :].rearrange("(t p) d -> p t d", p=P)
                for g in range(WPC // GH):
                    ws = g * GH
                    bnrs = []
                    for c in range(KC2):
                        bnr = bpool.tile([P, GH, P], f16, tag=f"bnr{c}")
                        nc.vector.tensor_scalar(
                            bnr[:], o1T[:, ws : ws + GH, c, :],
                            acT[:, c, 0:1], acT[:, c, 1:2],
                            mybir.AluOpType.mult, mybir.AluOpType.add,
                        )
                        nc.vector.tensor_scalar_max(bnr[:], bnr[:], 0.0)
                        bnrs.append(bnr)
                    for t in range(GH):
                        h2ps = h2pool.tile([P, D2], f32, tag="h2ps")
                        for c in range(KC2):
                            nc.tensor.matmul(
                                out=h2ps[:], lhsT=bnrs[c][:, t, :], rhs=w2_s[:, c, :],
                                start=(c == 0), stop=(c == KC2 - 1),
                            )
                        if (ws + t) % 4 == 3:
                            nc.vector.tensor_scalar_mul(
                                h2stage[:, ws + t, :], h2ps[:],
                                dismy_s[:, 1, ws + t : ws + t + 1],
                            )
                        else:
                            nc.scalar.activation(
                                h2stage[:, ws + t, :], h2ps[:], AF.Copy,
                                scale=dismy_s[:, 1, ws + t : ws + t + 1],
                            )
                    nc.sync.dma_start(
                        out=h2ap[:, ws : ws + GH, :],
                        in_=h2stage[:, ws : ws + GH, :],
                    )
            nc.gpsimd.collective_compute(
                "AllGather", mybir.AluOpType.bypass,
                ins=[h2shard[:, :]], outs=[h2tab[:, :]], replica_groups=RG,
            )
            if phases < 4:
                outst = cpool.tile([P, WPC, D2], f32)
                nc.vector.tensor_copy(out=outst[:], in_=h2stage[:])
                nc.gpsimd.dma_start(
                    out=out_d[0:SLOTS, :].rearrange("(t p) d -> p t d", p=P),
                    in_=outst[:],
                )
                return nc

            # ---------------- conv2 ----------------
            es1.close()  # free conv1 SBUF
            o2_pool = es.enter_context(tc.tile_pool(name="o2", bufs=1))
            o2_all = o2_pool.tile([P, WPC, D2], f16)
            lo2 = h2tab[0:HB, :]
            hi2 = h2tab[HB:NROWS, :]
            with (
                tc.tile_pool(name="g2", bufs=5) as gpool2,
                tc.tile_pool(name="sq2", bufs=6) as sqpool2,
                tc.tile_pool(name="c2p", bufs=6, space="PSUM") as wpool2,
                tc.tile_pool(name="st2p", bufs=1, space="PSUM") as stpool2,
            ):
                st2_s = stpool2.tile([1, D2], f32, tag="st2_s")
                st2_q = stpool2.tile([1, D2], f32, tag="st2_q")
                for w in range(WPC):
                    nl, nh = int(NL[w]), int(NH[w])
                    nt = nl + nh
                    gb = gpool2.tile([P, NTmax, D2], f16, tag="g2")
                    pieces = 3 if w == WPC - 1 else 2 if (w == WPC - 2 or w == 0) else 1
                    for n0, bs, view, off in ((nl, 0, lo2, offL[w]),
                                              (nh, nl, hi2, offH[w])):
                        if not n0:
                            continue
                        cuts = [n0 * i // pieces for i in range(pieces + 1)]
                        for a, b in zip(cuts[:-1], cuts[1:]):
                            if b == a:
                                continue
                            nc.gpsimd.dma_gather(
                                gb[:, bs + a : bs + b, :], view,
                                idx_s[:, off // 16 + a * 8 : off // 16 + b * 8],
                                (b - a) * P, (b - a) * P, D2,
                                elem_step=D2, single_packet=False,
                            )
                    ps = wpool2.tile([P, D2], f32, tag="win2")
                    # self-loop: h2stage row w IS dis*h2 for my slots
                    nc.tensor.matmul(
                        out=ps[:], lhsT=ident_b[:], rhs=h2stage[:, w, :],
                        start=True, stop=(nt == 0),
                    )
                    for j in range(nt):
                        nc.tensor.matmul(
                            out=ps[:], lhsT=ident_b[:], rhs=gb[:, j, :],
                            start=False, stop=(j == nt - 1),
                        )
                    nc.vector.tensor_scalar_mul(
                        o2_all[:, w, :], ps[:], dismy_s[:, 1, w : w + 1]
                    )
                    sq = sqpool2.tile([P, D2], f16, tag="sq2")
                    nc.vector.tensor_mul(sq[:], o2_all[:, w, :], o2_all[:, w, :])
                    mcol = mask_b[:, 1:2] if w == WPC - 1 else mask_b[:, 0:1]
                    nc.tensor.matmul(
                        out=st2_s[:], lhsT=mcol, rhs=o2_all[:, w, :],
                        start=(w == 0), stop=(w == WPC - 1), skip_group_check=True,
                    )
                    nc.tensor.matmul(
                        out=st2_q[:], lhsT=mcol, rhs=sq[:],
                        start=(w == 0), stop=(w == WPC - 1), skip_group_check=True,
                    )
                stats2 = o2_pool.tile([1, 2 * D2], f32)
                nc.vector.tensor_copy(out=stats2[:, 0:D2], in_=st2_s[:])
                nc.vector.tensor_copy(out=stats2[:, D2:], in_=st2_q[:])
            if phases < 5:
                outst = cpool.tile([P, WPC, D2], f32)
                nc.vector.tensor_copy(out=outst[:], in_=o2_all[:, :, :])
                nc.gpsimd.dma_start(
                    out=out_d[0:SLOTS, :].rearrange("(t p) d -> p t d", p=P),
                    in_=outst[:],
                )
                return nc
            nc.gpsimd.dma_start(out=ar2_in[:, :], in_=stats2[:])
            nc.gpsimd.collective_compute(
                "AllGather", mybir.AluOpType.bypass,
                ins=[ar2_in[:, :]], outs=[ar2_out[:, :]], replica_groups=RG,
            )

            # ---------------- BN2 + output ----------------
            sg28 = o2_pool.tile([N_CORES, 2 * D2], f32)
            nc.sync.dma_start(out=sg28[:], in_=ar2_out[:, :])
            ones82 = o2_pool.tile([N_CORES, 1], f32)
            nc.vector.memset(ones82[:], 1.0)
            mq2 = o2_pool.tile([1, 2 * D2], f32)
            with tc.tile_pool(name="sg2p", bufs=1, space="PSUM") as sg2pool:
                sg2ps = sg2pool.tile([1, 2 * D2], f32, tag="sg2ps")
                nc.tensor.matmul(out=sg2ps[:], lhsT=ones82[:], rhs=sg28[:], start=True, stop=True)
                nc.vector.tensor_scalar_mul(mq2[:], sg2ps[:], 1.0 / N)
            mean2 = mq2[:, 0:D2]
            ex22 = mq2[:, D2:]
            var2 = o2_pool.tile([1, D2], f32)
            nc.vector.tensor_mul(var2[:], mean2, mean2)
            nc.vector.tensor_sub(var2[:], ex22, var2[:])
            epst2 = o2_pool.tile([1, 1], f32)
            nc.vector.memset(epst2[:], EPS)
            sd2 = o2_pool.tile([1, D2], f32)
            nc.scalar.activation(sd2[:], var2[:], AF.Sqrt, bias=epst2[:])
            rstd2 = o2_pool.tile([1, D2], f32)
            nc.vector.reciprocal(rstd2[:], sd2[:])
            a2 = o2_pool.tile([1, D2], f32)
            nc.vector.tensor_mul(a2[:], rstd2[:], gb2_s[:, 0:D2])
            c2 = o2_pool.tile([1, D2], f32)
            nc.vector.tensor_mul(c2[:], mean2, a2[:])
            nc.vector.tensor_sub(c2[:], gb2_s[:, D2:], c2[:])

            # broadcast a2/c2 across partitions, tiled GW-wide
            GW = 7
            assert WPC % GW == 0
            onesrow = o2_pool.tile([1, P], f16)
            nc.vector.memset(onesrow[:], 1.0)
            ac2row = o2_pool.tile([1, 2, GW, D2], f16)
            for b, srct in ((0, a2), (1, c2)):
                nc.vector.tensor_copy(out=ac2row[:, b, 0, :], in_=srct[:])
                done = 1
                while done < GW:
                    n = min(done, GW - done)
                    nc.vector.tensor_copy(
                        out=ac2row[:, b, done : done + n, :],
                        in_=ac2row[:, b, 0:n, :],
                    )
                    done += n
            acb = o2_pool.tile([P, 2, GW, D2], f16)
            acb_flat = acb[:].rearrange("p b c d -> p (b c d)")
            ac2_flat = ac2row[:].rearrange("a b c d -> a (b c d)")
            CHK = 448  # <= 512 f32 per PSUM bank
            with tc.tile_pool(name="bn2p", bufs=4, space="PSUM") as bn2p:
                for i in range(0, 2 * GW * D2, CHK):
                    bps = bn2p.tile([P, CHK], f32, tag="b2a")
                    nc.tensor.matmul(out=bps[:], lhsT=onesrow[:],
                                     rhs=ac2_flat[:, i : i + CHK],
                                     start=True, stop=True)
                    nc.scalar.activation(acb_flat[:, i : i + CHK], bps[:], AF.Copy)

            outst = o2_pool.tile([P, WPC, D2], f16)
            outap = out_d[0:SLOTS, :].rearrange("(t p) d -> p t d", p=P)
            with tc.tile_pool(name="of32", bufs=8) as ofpool:
                ngroups = WPC // GW
                spans = [(g * GW, GW) for g in range(ngroups - 1)]
                spans += [((ngroups - 1) * GW + t, 1) for t in range(GW)]
                for s0, n in spans:
                    sl = slice(s0, s0 + n)
                    nc.vector.tensor_mul(outst[:, sl, :], o2_all[:, sl, :], acb[:, 0, 0:n])
                    nc.vector.tensor_add(outst[:, sl, :], outst[:, sl, :], acb[:, 1, 0:n])
                    of32 = ofpool.tile([P, GW, D2], f32, tag="of32")
                    nc.scalar.activation(of32[:, 0:n], outst[:, sl, :], AF.Copy)
                    nc.sync.dma_start(out=outap[:, sl, :], in_=of32[:, 0:n])

    return nc


# ---------------------------------------------------------------- entry point

def _run(x, edge_index, W1, gamma1, beta1, W2, gamma2, beta2, cfg, trace=False):
    from concourse.bass_utils import run_bass_kernel_spmd

    N = cfg["N"]
    pp = _preprocess(edge_index, N)
    xtab, xown, dismy2, w1b, w2b, g1t, b1t, i2 = _pack_inputs(
        np.asarray(x, np.float32), np.asarray(W1, np.float32),
        np.asarray(W2, np.float32), gamma1, beta1, pp, cfg)
    nc = _build_kernel(cfg, pp, phases=int(__import__("os").environ.get("K_PHASES", "5")))
    nc.compile()

    shared = {
        "xtab": np.ascontiguousarray(xtab),
        "i2": np.ascontiguousarray(i2),
        "w1b": np.ascontiguousarray(w1b),
        "w2b": np.ascontiguousarray(w2b),
        "g1t": np.ascontiguousarray(g1t),
        "b1t": np.ascontiguousarray(b1t),
        "statmask": np.ascontiguousarray(pp["statmask"]),
        "gamma2": np.asarray(gamma2, np.float32).reshape(1, -1),
        "beta2": np.asarray(beta2, np.float32).reshape(1, -1),
    }
    in_maps = []
    for c in range(N_CORES):
        m = dict(shared)
        m["idx"] = np.ascontiguousarray(pp["idx_wrapped"][c])
        m["xown"] = np.ascontiguousarray(xown[c])
        m["dismy"] = np.ascontiguousarray(dismy2[c])
        in_maps.append(m)

    res = run_bass_kernel_spmd(nc, in_maps, core_ids=list(range(N_CORES)), trace=trace)
    _run.last_nc = nc

    D2 = cfg["D2"]
    out = np.empty((N, D2), np.float32)
    pos, core_of = pp["pos"], pp["core_of"]
    for c in range(N_CORES):
        nodes = np.flatnonzero(core_of == c)
        out[nodes] = res.results[c]["out"][pos[nodes]]
    _run.last_result = res
    return out


def kernel(x, edge_index, W1, b1, gamma1, beta1, W2, b2, gamma2, beta2):
    # b1/b2 cancel exactly through BatchNorm's mean subtraction; unused.
    return _run(x, edge_index, W1, gamma1, beta1, W2, gamma2, beta2, _FULL_CFG)
